# revision 11
# baseline (speedup 1.0000x reference)
"""Trainium2 Bass kernel for the real-space Ewald sum (nn_Ewald).

Math (per molecule b, nb=2048 atoms, 8 charge channels):
    pot_b = sum_{i,j} qq_ij * erf(|rij|/sqrt(2)) / (|rij|+1e-6) / (4*pi)
          + sum_i qq_ii / (2*pi)^1.5            (self term)
    all scaled by NORM_FACTOR.

Key identity: w(s) = erf(sqrt(s/2))/sqrt(s) (s = |rij|^2) is analytic in s
near 0 and equals 1/sqrt(s) to <1e-3 for s > ~9.  So per pair tile:
    s'    = delta * s                 (K=13 bf16 hi/lo augmented matmul;
                                       delta = 2^-3 keeps products exact)
    y     = kappa/sqrt(s)             (one ACT pass: Abs_reciprocal_sqrt,
                                       scale = 1/(delta*kappa^2))
    w'    = min(y, max(P(s'), 1))     (ONE custom DVE op; P = monic cubic
                                       ((C0-s')s'+C1)s'+C2 fit to
                                       kappa*w(s) on s in [0, 9])
    u[c,j] += sum_i q_bf16[i,c] w'_ij (bf16 PE contraction, f32 PSUM)
Host: pot_b = (sum(u * q^T) - sum_i qq_ii*w'_ii)/kappa/(4*pi) + self term.
The diagonal is NOT masked on device; its value w'_ii = P(eps_ii) is
replicated on the host (eps_ii = the deterministic hi/lo rounding residue
of s'_ii) and subtracted exactly.  No erf table, no second ACT pass, no
second DVE pass, fully streaming (no phase barrier).

Symmetry: w is symmetric, so only the block upper triangle is computed.
Row block rb (0..15 within molecule, 128 atoms) covers col windows
jc >= rb//4; the diagonal 512-block gets weight 1 (both orientations of
in-group pairs are computed), strictly-above blocks get weight 2. Each
core takes 8 row blocks whose triangle widths form the multiset
{4,4,3,3,2,2,1,1} so all 8 cores run the identical program (SPMD).

Sharding: 8 cores = 4 molecules x 2 row-block sets.
"""

import numpy as np

B = 4
NB = 2048
NQ = 8
NCORES = 8
RB = 8              # row blocks per core
CT = 512            # matmul col tile (PSUM bank)
NORM_FACTOR = 90.0474
KA = 13             # augmented contraction depth (bf16 hi/lo split)

# fitted scheme constants (see fit in repo history): s_c=9, delta=2^-3
DELTA = 0.125
KAPPA = 3.0708577931200534
PC0 = 2.7432632222505378      # s0 -> C0
PC1 = -3.112066562880879      # s1 -> C1
PC2 = 2.4529603188601343      # imm2 -> C2
ACT_SCALE = 1.0 / (DELTA * KAPPA * KAPPA)

# per-slot triangle width in 512-blocks; identical multiset on every core
NJC = [4, 4, 3, 3, 2, 2, 1, 1]
# row blocks (within molecule) per core half
SLOT_RBG = {
    0: [0, 1, 4, 5, 8, 9, 12, 13],
    1: [2, 3, 6, 7, 10, 11, 14, 15],
}
# chunk widths per slot (PSUM tile granularity: 512 = 1 bank, 4-deep
# rotation keeps the A->ACT->DVE->A slot-recycle loop off the critical path)
CHUNKS = {4: [512] * 4, 3: [512] * 3, 2: [512] * 2, 1: [512]}
# last slot contributing to each u bank (first is always slot 0)
BANK_LAST_SLOT = {0: 1, 1: 3, 2: 5, 3: 7}

_compiled = None
_ops = None


def _register_ops():
    """Register the fused Ewald-weight DVE op (idempotent)."""
    global _ops
    if _ops is not None:
        return _ops
    from concourse import dve_ops
    from concourse.dve_spec import (
        Spec, Src0, Src1, C0, C1, C2, One, lower, _has_src1, minn, maxx,
    )
    from concourse.dve_uop import DveOpSpec

    def mk(name, spec):
        for o in dve_ops.OPS:
            if o.name == name:
                return o
        shas = {}
        for ver in ("v3", "v4"):
            tmp = DveOpSpec(
                name=name,
                opcode=31,
                uops=lower(spec, ver=ver),
                rd1_en=_has_src1(spec),
            )
            shas[ver] = tmp.sha(ver)
        op = dve_ops.DveOp(name, spec, subdim=False, uops_sha=shas)
        dve_ops.OPS.append(op)
        dve_ops._SUB_OPCODE_FOR_NAME[name] = (
            dve_ops._CUSTOM_DVE_ROW_BASE + len(dve_ops.OPS) - 1
        )
        dve_ops.CUSTOM_DVE_SPECS[name] = spec
        return op

    # w' = min(y, max(((C0 - s')s' + C1)s' + C2, 1))
    def _pw_ref(in0, in1, s0, s1, imm2):
        sp = in0.astype(np.float32)
        y = in1.astype(np.float32)
        p = ((np.float32(s0) - sp) * sp + np.float32(s1)) * sp + np.float32(imm2)
        return np.minimum(y, np.maximum(p, np.float32(1.0))).astype(np.float32)

    ewald_pw = mk(
        "EWALD_PW",
        Spec(
            body=minn(Src1, maxx(((C0 - Src0) * Src0 + C1) * Src0 + C2, One)),
            reference=_pw_ref,
        ),
    )
    _ops = (ewald_pw,)
    return _ops


def build_nc():
    """Build + compile the per-core Bass program (fixed shapes)."""
    from concourse import bacc, tile
    import concourse.mybir as mybir
    from concourse.bass import ts, ds

    (ewald_pw,) = _register_ops()
    f32 = mybir.dt.float32
    bf16 = mybir.dt.bfloat16
    AF = mybir.ActivationFunctionType

    nc = bacc.Bacc(
        "TRN2",
        target_bir_lowering=False,
        debug=False,
        num_devices=NCORES,
    )
    # aug split: slot-ordered stationary block and the full col block
    augs = nc.dram_tensor("augs", [KA, RB * 128], bf16, kind="ExternalInput").ap()
    augm = nc.dram_tensor("augm", [KA, NB], bf16, kind="ExternalInput").ap()
    # q12 = [q1 | q2] stacked on the free dim (one DMA)
    q12 = nc.dram_tensor(
        "q12", [128, 2 * RB * NQ], bf16, kind="ExternalInput"
    ).ap()
    uout = nc.dram_tensor("uout", [NQ, NB], f32, kind="ExternalOutput").ap()

    with tile.TileContext(nc) as tc:
        with (
            tc.tile_pool(name="const", bufs=1) as cpool,
            tc.tile_pool(name="work", bufs=4) as wpool,
            tc.tile_pool(name="ps", bufs=1, space="PSUM") as ps,
        ):
            # split input DMAs so the first chunk's operands land early;
            # keep the Scalar queue DMA-free so its ACT table load runs
            # during the DMA window.
            augs_sb = cpool.tile([KA, RB * 128], bf16)
            nc.sync.dma_start(out=augs_sb[:, 0:128], in_=augs[:, 0:128])
            augm_sb = cpool.tile([KA, NB], bf16)
            nc.sync.dma_start(out=augm_sb[:, 0:512], in_=augm[:, 0:512])
            nc.sync.dma_start(out=augm_sb[:, 512:NB], in_=augm[:, 512:NB])
            nc.sync.dma_start(
                out=augs_sb[:, 128 : RB * 128], in_=augs[:, 128 : RB * 128]
            )
            q12_sb = cpool.tile([128, 2 * RB * NQ], bf16)
            nc.gpsimd.dma_start(out=q12_sb[:], in_=q12[:])
            q1_sb = q12_sb[:, 0 : RB * NQ]
            q2_sb = q12_sb[:, RB * NQ : 2 * RB * NQ]

            # PSUM: 4 x [128,512] s tiles (4 banks) + [8,2048] u (4 banks)
            u_ps = ps.tile([NQ, 4 * CT], f32, tag="u", bufs=1)
            u_stage = {}

            for t in range(RB):
                col0 = NB - NJC[t] * CT  # window start col in molecule
                jc0 = 4 - NJC[t]         # first (diagonal) 512-block
                coff = 0
                for cw in CHUNKS[NJC[t]]:
                    s_ps = ps.tile([128, cw], f32, tag="s", bufs=4,
                                   padded_shape=[128, 512], name="s_ps")
                    for hminor in range(cw // CT):
                        nc.tensor.matmul(
                            s_ps[:, ts(hminor, CT)],
                            augs_sb[:, ts(t, 128)],
                            augm_sb[:, ds(col0 + coff + hminor * CT, CT)],
                            start=True,
                            stop=True,
                        )
                    y_sb = wpool.tile([128, cw], f32, tag="y",
                                      padded_shape=[128, 512], name="y_sb")
                    nc.scalar.activation(
                        y_sb[:], s_ps[:], AF.Abs_reciprocal_sqrt,
                        scale=ACT_SCALE,
                    )
                    w_sb = wpool.tile([128, cw], bf16, tag="w", bufs=8,
                                      padded_shape=[128, 512], name="w_sb")
                    nc.vector._custom_dve(
                        ewald_pw,
                        out=w_sb[:],
                        in0=s_ps[:],
                        in1=y_sb[:],
                        s0=PC0,
                        s1=PC1,
                        imm2=PC2,
                    )
                    for hminor in range(cw // CT):
                        jc = jc0 + (coff // CT) + hminor
                        qs = q1_sb if jc == jc0 else q2_sb
                        nc.tensor.matmul(
                            u_ps[:, ts(jc, CT)],
                            qs[:, ds(t * NQ, NQ)],
                            w_sb[:, ts(hminor, CT)],
                            start=(t == 0),
                            stop=(t == BANK_LAST_SLOT[jc]),
                        )
                    coff += cw
                # drain finished u banks early so the tail copy is short;
                # stage bank pairs and DMA once per pair
                for jc in range(4):
                    if BANK_LAST_SLOT[jc] == t:
                        pair = jc // 2
                        if jc % 2 == 0:
                            u_stage[pair] = wpool.tile(
                                [NQ, 2 * CT], f32, tag="u_sb", bufs=2,
                                name="u_sb",
                            )
                        nc.scalar.copy(
                            u_stage[pair][:, ts(jc % 2, CT)],
                            u_ps[:, ts(jc, CT)],
                        )
                        if jc % 2 == 1:
                            nc.gpsimd.dma_start(
                                out=uout[:, ds(pair * 2 * CT, 2 * CT)],
                                in_=u_stage[pair][:],
                            )

    nc.compile()
    return nc


def _make_mol(rm, qm):
    """Per-molecule hi/lo split (shared by in-map builder and host diag)."""
    import ml_dtypes

    bf = ml_dtypes.bfloat16
    rc = (rm - rm.mean(0, keepdims=True)).astype(np.float32)
    hi = rc.astype(bf)
    lo = (rc - hi.astype(np.float32)).astype(bf)
    rr = hi.astype(np.float32) + lo.astype(np.float32)
    n2 = (rr * rr).sum(1).astype(np.float32)
    n2_hi = n2.astype(bf)
    n2_lo = (n2 - n2_hi.astype(np.float32)).astype(bf)
    return hi, lo, n2_hi, n2_lo


def _aug_rows(hi, lo, n2_hi, n2_lo, rowsel):
    """The 13 (L, R) aug row pairs; R is scaled by DELTA (exact: 2^-3)."""
    import ml_dtypes

    bf = ml_dtypes.bfloat16
    n = hi.shape[0]
    dl = np.float32(DELTA)
    ones_i = np.ones(len(rowsel), bf)
    ones_j = np.full(n, dl, np.float32).astype(bf)  # delta exact in bf16
    rowsL, rowsR = [], []
    for ax in range(3):
        m2h = (-2.0 * hi[:, ax].astype(np.float32)).astype(bf)
        m2l = (-2.0 * lo[:, ax].astype(np.float32)).astype(bf)
        m2hd = (m2h.astype(np.float32) * dl).astype(bf)
        m2ld = (m2l.astype(np.float32) * dl).astype(bf)
        rowsL += [hi[rowsel, ax], hi[rowsel, ax], lo[rowsel, ax]]
        rowsR += [m2hd, m2ld, m2hd]
    n2_hid = (n2_hi.astype(np.float32) * dl).astype(bf)
    n2_lod = (n2_lo.astype(np.float32) * dl).astype(bf)
    rowsL += [n2_hi[rowsel], n2_lo[rowsel], ones_i, ones_i]
    rowsR += [ones_j, ones_j, n2_hid, n2_lod]
    return rowsL, rowsR


def make_in_maps(q, r):
    """Host-side sharding: per-core augmented bf16 hi/lo matrices."""
    import ml_dtypes

    bf = ml_dtypes.bfloat16
    q = np.ascontiguousarray(np.asarray(q, np.float32))
    r = np.ascontiguousarray(np.asarray(r, np.float32))
    in_maps = []
    for core in range(NCORES):
        b, h = core // 2, core % 2
        rm = r[b * NB : (b + 1) * NB]
        qm = q[b * NB : (b + 1) * NB]
        hi, lo, n2_hi, n2_lo = _make_mol(rm, qm)

        rbgs = SLOT_RBG[h]
        rowsel = np.concatenate(
            [np.arange(g * 128, (g + 1) * 128) for g in rbgs]
        )
        rowsL, rowsR = _aug_rows(hi, lo, n2_hi, n2_lo, rowsel)
        augs_np = np.ascontiguousarray(np.stack(rowsL).astype(bf))
        augm_np = np.ascontiguousarray(np.stack(rowsR).astype(bf))

        qi = qm[rowsel]  # [RB*128, NQ] slot-ordered
        q1_np = (
            qi.reshape(RB, 128, NQ).transpose(1, 0, 2).reshape(128, RB * NQ)
        ).astype(bf)
        q2_np = (2.0 * q1_np.astype(np.float32)).astype(bf)
        q12_np = np.ascontiguousarray(np.concatenate([q1_np, q2_np], axis=1))

        in_maps.append(
            {
                "augs": augs_np,
                "augm": augm_np,
                "q12": q12_np,
            }
        )
    return in_maps


def _host_diag_w(rm):
    """Replicate the device's diagonal weight w'_ii (f32 k-ordered accum)."""
    hi, lo, n2_hi, n2_lo = _make_mol(rm, None)
    rowsel = np.arange(NB)
    rowsL, rowsR = _aug_rows(hi, lo, n2_hi, n2_lo, rowsel)
    eps = np.zeros(NB, np.float32)
    for L, R in zip(rowsL, rowsR):
        eps = eps + L.astype(np.float32) * R.astype(np.float32)
    eps64 = eps.astype(np.float64)
    p = ((PC0 - eps64) * eps64 + PC1) * eps64 + PC2
    with np.errstate(divide="ignore"):
        y = 1.0 / np.sqrt(np.abs(ACT_SCALE * eps64))
    return np.minimum(y, np.maximum(p, 1.0))


def reduce_outputs(q, r, results):
    """Host-side gather: u[8,2048] per core -> pot[B].

    The device computes the diagonal with weight w'_ii = P(eps_ii)
    (eps_ii = deterministic rounding residue of s'_ii); replicate it
    here and subtract, then add the self term in f64.
    """
    q = np.asarray(q, np.float32)
    r = np.asarray(r, np.float32)
    TWOPI = 2.0 * np.pi
    pots = np.zeros(B, np.float64)
    for core in range(NCORES):
        b = core // 2
        u = results[core]["uout"].astype(np.float64)
        qm = q[b * NB : (b + 1) * NB].astype(np.float64)
        pots[b] += (u * qm.T).sum()
    for b in range(B):
        rm = r[b * NB : (b + 1) * NB]
        qm = q[b * NB : (b + 1) * NB].astype(np.float64)
        wdiag = _host_diag_w(rm)
        pots[b] -= ((qm**2).sum(1) * wdiag).sum()
    pots = pots / KAPPA / (4.0 * np.pi)
    for b in range(B):
        qm = q[b * NB : (b + 1) * NB].astype(np.float64)
        pots[b] += (qm**2).sum() / ((2.0 * np.pi) ** 1.5)
    return (pots * NORM_FACTOR).astype(np.float32)


def kernel(q, r, batch):
    global _compiled
    if _compiled is None:
        _compiled = build_nc()
    from concourse import bass_utils

    in_maps = make_in_maps(q, r)
    last_err = None
    for attempt in range(3):
        try:
            res = bass_utils.run_bass_kernel_spmd(
                _compiled, in_maps, core_ids=list(range(NCORES))
            )
            return reduce_outputs(q, r, res.results)
        except Exception as e:  # transient device errors: back off and retry
            last_err = e
            import time

            time.sleep(15 * (attempt + 1))
    raise last_err


# revision 13
# speedup vs baseline: 1.0083x; 1.0083x over previous
"""Trainium2 Bass kernel for the real-space Ewald sum (nn_Ewald).

Math (per molecule b, nb=2048 atoms, 8 charge channels):
    pot_b = sum_{i,j} qq_ij * erf(|rij|/sqrt(2)) / (|rij|+1e-6) / (4*pi)
          + sum_i qq_ii / (2*pi)^1.5            (self term)
    all scaled by NORM_FACTOR.

Key identity: w(s) = erf(sqrt(s/2))/sqrt(s) (s = |rij|^2) is analytic in s
near 0 and equals 1/sqrt(s) to <1e-3 for s > ~9.  So per pair tile:
    s'    = delta * s                 (K=13 bf16 hi/lo augmented matmul;
                                       delta = 2^-3 keeps products exact)
    y     = kappa/sqrt(s)             (one ACT pass: Abs_reciprocal_sqrt,
                                       scale = 1/(delta*kappa^2))
    w'    = min(y, max(P(s'), 1))     (ONE custom DVE op; P = monic cubic
                                       ((C0-s')s'+C1)s'+C2 fit to
                                       kappa*w(s) on s in [0, 9])
    u[c,j] += sum_i q_bf16[i,c] w'_ij (bf16 PE contraction, f32 PSUM)
Host: pot_b = (sum(u * q^T) - sum_i qq_ii*w'_ii)/kappa/(4*pi) + self term.
The diagonal is NOT masked on device; its value w'_ii = P(eps_ii) is
replicated on the host (eps_ii = the deterministic hi/lo rounding residue
of s'_ii) and subtracted exactly.  No erf table, no second ACT pass, no
second DVE pass, fully streaming (no phase barrier).

Symmetry: w is symmetric, so only the block upper triangle is computed.
Row block rb (0..15 within molecule, 128 atoms) covers col windows
jc >= rb//4; the diagonal 512-block gets weight 1 (both orientations of
in-group pairs are computed), strictly-above blocks get weight 2. Each
core takes 8 row blocks whose triangle widths form the multiset
{4,4,3,3,2,2,1,1} so all 8 cores run the identical program (SPMD).

Sharding: 8 cores = 4 molecules x 2 row-block sets.
"""

import numpy as np

B = 4
NB = 2048
NQ = 8
NCORES = 8
RB = 8              # row blocks per core
CT = 512            # matmul col tile (PSUM bank)
NORM_FACTOR = 90.0474
KA = 13             # augmented contraction depth (bf16 hi/lo split)

# fitted scheme constants (see fit in repo history): s_c=9, delta=2^-3
DELTA = 0.125
KAPPA = 3.0708577931200534
PC0 = 2.7432632222505378      # s0 -> C0
PC1 = -3.112066562880879      # s1 -> C1
PC2 = 2.4529603188601343      # imm2 -> C2
ACT_SCALE = 1.0 / (DELTA * KAPPA * KAPPA)

# per-slot triangle width in 512-blocks; identical multiset on every core
NJC = [4, 4, 3, 3, 2, 2, 1, 1]
# row blocks (within molecule) per core half
SLOT_RBG = {
    0: [0, 1, 4, 5, 8, 9, 12, 13],
    1: [2, 3, 6, 7, 10, 11, 14, 15],
}
# chunk widths per slot (PSUM tile granularity: 512 = 1 bank, 4-deep
# rotation keeps the A->ACT->DVE->A slot-recycle loop off the critical path)
CHUNKS = {4: [512] * 4, 3: [512] * 3, 2: [512] * 2, 1: [512]}
# last slot contributing to each u bank (first is always slot 0)
BANK_LAST_SLOT = {0: 1, 1: 3, 2: 5, 3: 7}

_compiled = None
_ops = None


def _register_ops():
    """Register the fused Ewald-weight DVE op (idempotent)."""
    global _ops
    if _ops is not None:
        return _ops
    from concourse import dve_ops
    from concourse.dve_spec import (
        Spec, Src0, Src1, C0, C1, C2, One, lower, _has_src1, minn, maxx,
    )
    from concourse.dve_uop import DveOpSpec

    def mk(name, spec):
        for o in dve_ops.OPS:
            if o.name == name:
                return o
        shas = {}
        for ver in ("v3", "v4"):
            tmp = DveOpSpec(
                name=name,
                opcode=31,
                uops=lower(spec, ver=ver),
                rd1_en=_has_src1(spec),
            )
            shas[ver] = tmp.sha(ver)
        op = dve_ops.DveOp(name, spec, subdim=False, uops_sha=shas)
        dve_ops.OPS.append(op)
        dve_ops._SUB_OPCODE_FOR_NAME[name] = (
            dve_ops._CUSTOM_DVE_ROW_BASE + len(dve_ops.OPS) - 1
        )
        dve_ops.CUSTOM_DVE_SPECS[name] = spec
        return op

    # w' = min(y, max(((C0 - s')s' + C1)s' + C2, 1))
    def _pw_ref(in0, in1, s0, s1, imm2):
        sp = in0.astype(np.float32)
        y = in1.astype(np.float32)
        p = ((np.float32(s0) - sp) * sp + np.float32(s1)) * sp + np.float32(imm2)
        return np.minimum(y, np.maximum(p, np.float32(1.0))).astype(np.float32)

    ewald_pw = mk(
        "EWALD_PW",
        Spec(
            body=minn(Src1, maxx(((C0 - Src0) * Src0 + C1) * Src0 + C2, One)),
            reference=_pw_ref,
        ),
    )
    _ops = (ewald_pw,)
    return _ops


def build_nc():
    """Build + compile the per-core Bass program (fixed shapes)."""
    from concourse import bacc, tile
    import concourse.mybir as mybir
    from concourse.bass import ts, ds

    (ewald_pw,) = _register_ops()
    f32 = mybir.dt.float32
    bf16 = mybir.dt.bfloat16
    AF = mybir.ActivationFunctionType

    nc = bacc.Bacc(
        "TRN2",
        target_bir_lowering=False,
        debug=False,
        num_devices=NCORES,
    )
    # aug split: slot-ordered stationary block and the full col block
    augs = nc.dram_tensor("augs", [KA, RB * 128], bf16, kind="ExternalInput").ap()
    augm = nc.dram_tensor("augm", [KA, NB], bf16, kind="ExternalInput").ap()
    # q12 = [q1 | q2] stacked on the free dim (one DMA)
    q12 = nc.dram_tensor(
        "q12", [128, 2 * RB * NQ], bf16, kind="ExternalInput"
    ).ap()
    uout = nc.dram_tensor("uout", [NQ, NB], f32, kind="ExternalOutput").ap()

    with tile.TileContext(nc) as tc:
        with (
            tc.tile_pool(name="const", bufs=1) as cpool,
            tc.tile_pool(name="work", bufs=4) as wpool,
            tc.tile_pool(name="ps", bufs=1, space="PSUM") as ps,
        ):
            # split input DMAs; first chunk's operands issued in parallel
            # on different queues so they land early.
            augm_sb = cpool.tile([KA, NB], bf16)
            nc.sync.dma_start(out=augm_sb[:, 0:512], in_=augm[:, 0:512])
            augs_sb = cpool.tile([KA, RB * 128], bf16)
            nc.scalar.dma_start(out=augs_sb[:, 0:128], in_=augs[:, 0:128])
            nc.sync.dma_start(out=augm_sb[:, 512:NB], in_=augm[:, 512:NB])
            nc.scalar.dma_start(
                out=augs_sb[:, 128 : RB * 128], in_=augs[:, 128 : RB * 128]
            )
            q12_sb = cpool.tile([128, 2 * RB * NQ], bf16)
            nc.gpsimd.dma_start(out=q12_sb[:], in_=q12[:])
            q1_sb = q12_sb[:, 0 : RB * NQ]
            q2_sb = q12_sb[:, RB * NQ : 2 * RB * NQ]

            # PSUM: 4 x [128,512] s tiles (4 banks) + [8,2048] u (4 banks)
            u_ps = ps.tile([NQ, 4 * CT], f32, tag="u", bufs=1)
            u_stage = {}

            # flat chunk list: (slot t, jc, col) with cw == CT always
            chunk_list = []
            for t in range(RB):
                col0 = NB - NJC[t] * CT  # window start col in molecule
                jc0 = 4 - NJC[t]         # first (diagonal) 512-block
                for ci in range(NJC[t]):
                    chunk_list.append((t, jc0 + ci, col0 + ci * CT, jc0))
            n_chunks = len(chunk_list)

            def emit_b(k, w_tile):
                """Phase-B matmul for chunk k (+ u drain when a bank ends)."""
                t, jc, _, jc0 = chunk_list[k]
                qs = q1_sb if jc == jc0 else q2_sb
                nc.tensor.matmul(
                    u_ps[:, ts(jc, CT)],
                    qs[:, ds(t * NQ, NQ)],
                    w_tile[:],
                    start=(t == 0),
                    stop=(t == BANK_LAST_SLOT[jc]),
                )
                last_of_slot = k + 1 == n_chunks or chunk_list[k + 1][0] != t
                if not last_of_slot:
                    return
                for jb in range(4):
                    if BANK_LAST_SLOT[jb] == t:
                        pair = jb // 2
                        if jb % 2 == 0:
                            u_stage[pair] = wpool.tile(
                                [NQ, 2 * CT], f32, tag="u_sb", bufs=2,
                                name="u_sb",
                            )
                        nc.scalar.copy(
                            u_stage[pair][:, ts(jb % 2, CT)],
                            u_ps[:, ts(jb, CT)],
                        )
                        if jb % 2 == 1:
                            nc.gpsimd.dma_start(
                                out=uout[:, ds(pair * 2 * CT, 2 * CT)],
                                in_=u_stage[pair][:],
                            )

            LAG = 2  # phase-B trails by LAG chunks so it never stalls tensor
            w_tiles = {}
            for k, (t, jc, col, jc0) in enumerate(chunk_list):
                s_ps = ps.tile([128, CT], f32, tag="s", bufs=4, name="s_ps")
                nc.tensor.matmul(
                    s_ps[:],
                    augs_sb[:, ts(t, 128)],
                    augm_sb[:, ds(col, CT)],
                    start=True,
                    stop=True,
                )
                y_sb = wpool.tile([128, CT], f32, tag="y", name="y_sb")
                nc.scalar.activation(
                    y_sb[:], s_ps[:], AF.Abs_reciprocal_sqrt,
                    scale=ACT_SCALE,
                )
                w_sb = wpool.tile([128, CT], bf16, tag="w", bufs=8,
                                  name="w_sb")
                nc.vector._custom_dve(
                    ewald_pw,
                    out=w_sb[:],
                    in0=s_ps[:],
                    in1=y_sb[:],
                    s0=PC0,
                    s1=PC1,
                    imm2=PC2,
                )
                w_tiles[k] = w_sb
                if k >= LAG:
                    emit_b(k - LAG, w_tiles.pop(k - LAG))
            for k in range(n_chunks - LAG, n_chunks):
                emit_b(k, w_tiles.pop(k))

    nc.compile()
    return nc


def _make_mol(rm, qm):
    """Per-molecule hi/lo split (shared by in-map builder and host diag)."""
    import ml_dtypes

    bf = ml_dtypes.bfloat16
    rc = (rm - rm.mean(0, keepdims=True)).astype(np.float32)
    hi = rc.astype(bf)
    lo = (rc - hi.astype(np.float32)).astype(bf)
    rr = hi.astype(np.float32) + lo.astype(np.float32)
    n2 = (rr * rr).sum(1).astype(np.float32)
    n2_hi = n2.astype(bf)
    n2_lo = (n2 - n2_hi.astype(np.float32)).astype(bf)
    return hi, lo, n2_hi, n2_lo


def _aug_rows(hi, lo, n2_hi, n2_lo, rowsel):
    """The 13 (L, R) aug row pairs; R is scaled by DELTA (exact: 2^-3)."""
    import ml_dtypes

    bf = ml_dtypes.bfloat16
    n = hi.shape[0]
    dl = np.float32(DELTA)
    ones_i = np.ones(len(rowsel), bf)
    ones_j = np.full(n, dl, np.float32).astype(bf)  # delta exact in bf16
    rowsL, rowsR = [], []
    for ax in range(3):
        m2h = (-2.0 * hi[:, ax].astype(np.float32)).astype(bf)
        m2l = (-2.0 * lo[:, ax].astype(np.float32)).astype(bf)
        m2hd = (m2h.astype(np.float32) * dl).astype(bf)
        m2ld = (m2l.astype(np.float32) * dl).astype(bf)
        rowsL += [hi[rowsel, ax], hi[rowsel, ax], lo[rowsel, ax]]
        rowsR += [m2hd, m2ld, m2hd]
    n2_hid = (n2_hi.astype(np.float32) * dl).astype(bf)
    n2_lod = (n2_lo.astype(np.float32) * dl).astype(bf)
    rowsL += [n2_hi[rowsel], n2_lo[rowsel], ones_i, ones_i]
    rowsR += [ones_j, ones_j, n2_hid, n2_lod]
    return rowsL, rowsR


def make_in_maps(q, r):
    """Host-side sharding: per-core augmented bf16 hi/lo matrices."""
    import ml_dtypes

    bf = ml_dtypes.bfloat16
    q = np.ascontiguousarray(np.asarray(q, np.float32))
    r = np.ascontiguousarray(np.asarray(r, np.float32))
    in_maps = []
    for core in range(NCORES):
        b, h = core // 2, core % 2
        rm = r[b * NB : (b + 1) * NB]
        qm = q[b * NB : (b + 1) * NB]
        hi, lo, n2_hi, n2_lo = _make_mol(rm, qm)

        rbgs = SLOT_RBG[h]
        rowsel = np.concatenate(
            [np.arange(g * 128, (g + 1) * 128) for g in rbgs]
        )
        rowsL, rowsR = _aug_rows(hi, lo, n2_hi, n2_lo, rowsel)
        augs_np = np.ascontiguousarray(np.stack(rowsL).astype(bf))
        augm_np = np.ascontiguousarray(np.stack(rowsR).astype(bf))

        qi = qm[rowsel]  # [RB*128, NQ] slot-ordered
        q1_np = (
            qi.reshape(RB, 128, NQ).transpose(1, 0, 2).reshape(128, RB * NQ)
        ).astype(bf)
        q2_np = (2.0 * q1_np.astype(np.float32)).astype(bf)
        q12_np = np.ascontiguousarray(np.concatenate([q1_np, q2_np], axis=1))

        in_maps.append(
            {
                "augs": augs_np,
                "augm": augm_np,
                "q12": q12_np,
            }
        )
    return in_maps


def _host_diag_w(rm):
    """Replicate the device's diagonal weight w'_ii (f32 k-ordered accum)."""
    hi, lo, n2_hi, n2_lo = _make_mol(rm, None)
    rowsel = np.arange(NB)
    rowsL, rowsR = _aug_rows(hi, lo, n2_hi, n2_lo, rowsel)
    eps = np.zeros(NB, np.float32)
    for L, R in zip(rowsL, rowsR):
        eps = eps + L.astype(np.float32) * R.astype(np.float32)
    eps64 = eps.astype(np.float64)
    p = ((PC0 - eps64) * eps64 + PC1) * eps64 + PC2
    with np.errstate(divide="ignore"):
        y = 1.0 / np.sqrt(np.abs(ACT_SCALE * eps64))
    return np.minimum(y, np.maximum(p, 1.0))


def reduce_outputs(q, r, results):
    """Host-side gather: u[8,2048] per core -> pot[B].

    The device computes the diagonal with weight w'_ii = P(eps_ii)
    (eps_ii = deterministic rounding residue of s'_ii); replicate it
    here and subtract, then add the self term in f64.
    """
    q = np.asarray(q, np.float32)
    r = np.asarray(r, np.float32)
    TWOPI = 2.0 * np.pi
    pots = np.zeros(B, np.float64)
    for core in range(NCORES):
        b = core // 2
        u = results[core]["uout"].astype(np.float64)
        qm = q[b * NB : (b + 1) * NB].astype(np.float64)
        pots[b] += (u * qm.T).sum()
    for b in range(B):
        rm = r[b * NB : (b + 1) * NB]
        qm = q[b * NB : (b + 1) * NB].astype(np.float64)
        wdiag = _host_diag_w(rm)
        pots[b] -= ((qm**2).sum(1) * wdiag).sum()
    pots = pots / KAPPA / (4.0 * np.pi)
    for b in range(B):
        qm = q[b * NB : (b + 1) * NB].astype(np.float64)
        pots[b] += (qm**2).sum() / ((2.0 * np.pi) ** 1.5)
    return (pots * NORM_FACTOR).astype(np.float32)


def kernel(q, r, batch):
    global _compiled
    if _compiled is None:
        _compiled = build_nc()
    from concourse import bass_utils

    in_maps = make_in_maps(q, r)
    last_err = None
    for attempt in range(3):
        try:
            res = bass_utils.run_bass_kernel_spmd(
                _compiled, in_maps, core_ids=list(range(NCORES))
            )
            return reduce_outputs(q, r, res.results)
        except Exception as e:  # transient device errors: back off and retry
            last_err = e
            import time

            time.sleep(15 * (attempt + 1))
    raise last_err


# revision 16
# speedup vs baseline: 1.0298x; 1.0213x over previous
"""Trainium2 Bass kernel for the real-space Ewald sum (nn_Ewald).

Math (per molecule b, nb=2048 atoms, 8 charge channels):
    pot_b = sum_{i,j} qq_ij * erf(|rij|/sqrt(2)) / (|rij|+1e-6) / (4*pi)
          + sum_i qq_ii / (2*pi)^1.5            (self term)
    all scaled by NORM_FACTOR.

Key identity: w(s) = erf(sqrt(s/2))/sqrt(s) (s = |rij|^2) is analytic in s
near 0 and equals 1/sqrt(s) to <1e-3 for s > ~9.  So per pair tile:
    s'    = delta * s                 (K=13 bf16 hi/lo augmented matmul;
                                       delta = 2^-3 keeps products exact)
    y     = kappa/sqrt(s)             (one ACT pass: Abs_reciprocal_sqrt,
                                       scale = 1/(delta*kappa^2))
    w'    = min(y, max(P(s'), 1))     (ONE custom DVE op; P = monic cubic
                                       ((C0-s')s'+C1)s'+C2 fit to
                                       kappa*w(s) on s in [0, 9])
    u[c,j] += sum_i q_bf16[i,c] w'_ij (bf16 PE contraction, f32 PSUM)
Host: pot_b = (sum(u * q^T) - sum_i qq_ii*w'_ii)/kappa/(4*pi) + self term.
The diagonal is NOT masked on device; its value w'_ii = P(eps_ii) is
replicated on the host (eps_ii = the deterministic hi/lo rounding residue
of s'_ii) and subtracted exactly.  No erf table, no second ACT pass, no
second DVE pass, fully streaming (no phase barrier).

Symmetry: w is symmetric, so only the block upper triangle is computed.
Row block rb (0..15 within molecule, 128 atoms) covers col windows
jc >= rb//4; the diagonal 512-block gets weight 1 (both orientations of
in-group pairs are computed), strictly-above blocks get weight 2. Each
core takes 8 row blocks whose triangle widths form the multiset
{4,4,3,3,2,2,1,1} so all 8 cores run the identical program (SPMD).

Sharding: 8 cores = 4 molecules x 2 row-block sets.
"""

import numpy as np

B = 4
NB = 2048
NQ = 8
NCORES = 8
RB = 8              # row blocks per core
CT = 512            # matmul col tile (PSUM bank)
NORM_FACTOR = 90.0474
KA = 13             # augmented contraction depth (bf16 hi/lo split)

# fitted scheme constants (see fit in repo history): s_c=9, delta=2^-3
DELTA = 0.125
KAPPA = 3.0708577931200534
PC0 = 2.7432632222505378      # s0 -> C0
PC1 = -3.112066562880879      # s1 -> C1
PC2 = 2.4529603188601343      # imm2 -> C2
ACT_SCALE = 1.0 / (DELTA * KAPPA * KAPPA)

# per-slot triangle width in 512-blocks; identical multiset on every core
NJC = [4, 4, 3, 3, 2, 2, 1, 1]
# row blocks (within molecule) per core half
SLOT_RBG = {
    0: [0, 1, 4, 5, 8, 9, 12, 13],
    1: [2, 3, 6, 7, 10, 11, 14, 15],
}
# chunk widths per slot (PSUM tile granularity: 512 = 1 bank, 4-deep
# rotation keeps the A->ACT->DVE->A slot-recycle loop off the critical path)
CHUNKS = {4: [512] * 4, 3: [512] * 3, 2: [512] * 2, 1: [512]}
# last slot contributing to each u bank (first is always slot 0)
BANK_LAST_SLOT = {0: 1, 1: 3, 2: 5, 3: 7}

_compiled = None
_ops = None


def _register_ops():
    """Register the fused Ewald-weight DVE op (idempotent)."""
    global _ops
    if _ops is not None:
        return _ops
    from concourse import dve_ops
    from concourse.dve_spec import (
        Spec, Src0, Src1, C0, C1, C2, One, lower, _has_src1, minn, maxx,
    )
    from concourse.dve_uop import DveOpSpec

    def mk(name, spec):
        for o in dve_ops.OPS:
            if o.name == name:
                return o
        shas = {}
        for ver in ("v3", "v4"):
            tmp = DveOpSpec(
                name=name,
                opcode=31,
                uops=lower(spec, ver=ver),
                rd1_en=_has_src1(spec),
            )
            shas[ver] = tmp.sha(ver)
        op = dve_ops.DveOp(name, spec, subdim=False, uops_sha=shas)
        dve_ops.OPS.append(op)
        dve_ops._SUB_OPCODE_FOR_NAME[name] = (
            dve_ops._CUSTOM_DVE_ROW_BASE + len(dve_ops.OPS) - 1
        )
        dve_ops.CUSTOM_DVE_SPECS[name] = spec
        return op

    # w' = min(y, max(((C0 - s')s' + C1)s' + C2, 1))
    def _pw_ref(in0, in1, s0, s1, imm2):
        sp = in0.astype(np.float32)
        y = in1.astype(np.float32)
        p = ((np.float32(s0) - sp) * sp + np.float32(s1)) * sp + np.float32(imm2)
        return np.minimum(y, np.maximum(p, np.float32(1.0))).astype(np.float32)

    ewald_pw = mk(
        "EWALD_PW",
        Spec(
            body=minn(Src1, maxx(((C0 - Src0) * Src0 + C1) * Src0 + C2, One)),
            reference=_pw_ref,
        ),
    )
    _ops = (ewald_pw,)
    return _ops


def build_nc():
    """Build + compile the per-core Bass program (fixed shapes)."""
    from concourse import bacc, tile
    import concourse.mybir as mybir
    from concourse.bass import ts, ds

    (ewald_pw,) = _register_ops()
    f32 = mybir.dt.float32
    bf16 = mybir.dt.bfloat16
    AF = mybir.ActivationFunctionType

    nc = bacc.Bacc(
        "TRN2",
        target_bir_lowering=False,
        debug=False,
        num_devices=NCORES,
    )
    # aug split: slot-ordered stationary block and the full col block
    augs = nc.dram_tensor("augs", [KA, RB * 128], bf16, kind="ExternalInput").ap()
    augm = nc.dram_tensor("augm", [KA, NB], bf16, kind="ExternalInput").ap()
    # q12 = [q1 | q2] stacked on the free dim (one DMA)
    q12 = nc.dram_tensor(
        "q12", [128, 2 * RB * NQ], bf16, kind="ExternalInput"
    ).ap()
    uout = nc.dram_tensor("uout", [NQ, NB], f32, kind="ExternalOutput").ap()

    with tile.TileContext(nc) as tc:
        with (
            tc.tile_pool(name="const", bufs=1) as cpool,
            tc.tile_pool(name="work", bufs=4) as wpool,
            tc.tile_pool(name="ps", bufs=1, space="PSUM") as ps,
        ):
            # split input DMAs; first chunk's operands issued in parallel
            # on different queues so they land early.
            augm_sb = cpool.tile([KA, NB], bf16)
            nc.sync.dma_start(out=augm_sb[:, 0:512], in_=augm[:, 0:512])
            augs_sb = cpool.tile([KA, RB * 128], bf16)
            nc.scalar.dma_start(out=augs_sb[:, 0:128], in_=augs[:, 0:128])
            nc.sync.dma_start(out=augm_sb[:, 512:NB], in_=augm[:, 512:NB])
            nc.scalar.dma_start(
                out=augs_sb[:, 128 : RB * 128], in_=augs[:, 128 : RB * 128]
            )
            q12_sb = cpool.tile([128, 2 * RB * NQ], bf16)
            nc.gpsimd.dma_start(out=q12_sb[:], in_=q12[:])
            q1_sb = q12_sb[:, 0 : RB * NQ]
            q2_sb = q12_sb[:, RB * NQ : 2 * RB * NQ]

            # PSUM: 4 x [128,512] s tiles (4 banks) + [8,2048] u (4 banks)
            u_ps = ps.tile([NQ, 4 * CT], f32, tag="u", bufs=1)
            u_stage = {}

            # flat chunk list: (slot t, jc, col) with cw == CT always
            chunk_list = []
            for t in range(RB):
                col0 = NB - NJC[t] * CT  # window start col in molecule
                jc0 = 4 - NJC[t]         # first (diagonal) 512-block
                for ci in range(NJC[t]):
                    chunk_list.append((t, jc0 + ci, col0 + ci * CT, jc0))
            n_chunks = len(chunk_list)

            def emit_b(k, w_tile):
                """Phase-B matmul for chunk k."""
                t, jc, _, jc0 = chunk_list[k]
                qs = q1_sb if jc == jc0 else q2_sb
                nc.tensor.matmul(
                    u_ps[:, ts(jc, CT)],
                    qs[:, ds(t * NQ, NQ)],
                    w_tile[:],
                    start=(t == 0),
                    stop=(t == BANK_LAST_SLOT[jc]),
                )

            def drain_u(t):
                """Copy + DMA any u bank whose accumulation ended at slot t."""
                for jb in range(4):
                    if BANK_LAST_SLOT[jb] == t:
                        pair = jb // 2
                        if jb % 2 == 0:
                            u_stage[pair] = wpool.tile(
                                [NQ, 2 * CT], f32, tag="u_sb", bufs=2,
                                name="u_sb",
                            )
                        nc.scalar.copy(
                            u_stage[pair][:, ts(jb % 2, CT)],
                            u_ps[:, ts(jb, CT)],
                        )
                        if jb % 2 == 1:
                            nc.gpsimd.dma_start(
                                out=uout[:, ds(pair * 2 * CT, 2 * CT)],
                                in_=u_stage[pair][:],
                            )

            # Slot-batched emission: all A-matmuls of slot t share one
            # stationary (augs[t]) and run back-to-back (same-weight
            # LDWEIGHTS are free); then the lagging B-group of slot t-1
            # runs q1-chunk first, q2-chunks contiguously — 3 weight
            # swaps per slot instead of 2 per chunk.
            w_tiles = {}
            slot_chunks = {}
            for k, (t, jc, col, jc0) in enumerate(chunk_list):
                slot_chunks.setdefault(t, []).append(k)

            def emit_b_group(t):
                ks = slot_chunks[t]
                # q1 (diagonal) chunk first, then q2 chunks
                for k in sorted(ks, key=lambda k: chunk_list[k][1] != chunk_list[k][3]):
                    emit_b(k, w_tiles.pop(k))
                drain_u(t)

            for t in range(RB):
                for k in slot_chunks[t]:
                    _, jc, col, jc0 = chunk_list[k]
                    s_ps = ps.tile([128, CT], f32, tag="s", bufs=4,
                                   name="s_ps")
                    nc.tensor.matmul(
                        s_ps[:],
                        augs_sb[:, ts(t, 128)],
                        augm_sb[:, ds(col, CT)],
                        start=True,
                        stop=True,
                    )
                    y_sb = wpool.tile([128, CT], f32, tag="y", name="y_sb")
                    nc.scalar.activation(
                        y_sb[:], s_ps[:], AF.Abs_reciprocal_sqrt,
                        scale=ACT_SCALE,
                    )
                    w_sb = wpool.tile([128, CT], bf16, tag="w", bufs=8,
                                      name="w_sb")
                    nc.vector._custom_dve(
                        ewald_pw,
                        out=w_sb[:],
                        in0=s_ps[:],
                        in1=y_sb[:],
                        s0=PC0,
                        s1=PC1,
                        imm2=PC2,
                    )
                    w_tiles[k] = w_sb
                if t > 0:
                    emit_b_group(t - 1)
            emit_b_group(RB - 1)

    nc.compile()
    return nc


def _make_mol(rm, qm):
    """Per-molecule hi/lo split (shared by in-map builder and host diag)."""
    import ml_dtypes

    bf = ml_dtypes.bfloat16
    rc = (rm - rm.mean(0, keepdims=True)).astype(np.float32)
    hi = rc.astype(bf)
    lo = (rc - hi.astype(np.float32)).astype(bf)
    rr = hi.astype(np.float32) + lo.astype(np.float32)
    n2 = (rr * rr).sum(1).astype(np.float32)
    n2_hi = n2.astype(bf)
    n2_lo = (n2 - n2_hi.astype(np.float32)).astype(bf)
    return hi, lo, n2_hi, n2_lo


def _aug_rows(hi, lo, n2_hi, n2_lo, rowsel):
    """The 13 (L, R) aug row pairs; R is scaled by DELTA (exact: 2^-3)."""
    import ml_dtypes

    bf = ml_dtypes.bfloat16
    n = hi.shape[0]
    dl = np.float32(DELTA)
    ones_i = np.ones(len(rowsel), bf)
    ones_j = np.full(n, dl, np.float32).astype(bf)  # delta exact in bf16
    rowsL, rowsR = [], []
    for ax in range(3):
        m2h = (-2.0 * hi[:, ax].astype(np.float32)).astype(bf)
        m2l = (-2.0 * lo[:, ax].astype(np.float32)).astype(bf)
        m2hd = (m2h.astype(np.float32) * dl).astype(bf)
        m2ld = (m2l.astype(np.float32) * dl).astype(bf)
        rowsL += [hi[rowsel, ax], hi[rowsel, ax], lo[rowsel, ax]]
        rowsR += [m2hd, m2ld, m2hd]
    n2_hid = (n2_hi.astype(np.float32) * dl).astype(bf)
    n2_lod = (n2_lo.astype(np.float32) * dl).astype(bf)
    rowsL += [n2_hi[rowsel], n2_lo[rowsel], ones_i, ones_i]
    rowsR += [ones_j, ones_j, n2_hid, n2_lod]
    return rowsL, rowsR


def make_in_maps(q, r):
    """Host-side sharding: per-core augmented bf16 hi/lo matrices."""
    import ml_dtypes

    bf = ml_dtypes.bfloat16
    q = np.ascontiguousarray(np.asarray(q, np.float32))
    r = np.ascontiguousarray(np.asarray(r, np.float32))
    in_maps = []
    for core in range(NCORES):
        b, h = core // 2, core % 2
        rm = r[b * NB : (b + 1) * NB]
        qm = q[b * NB : (b + 1) * NB]
        hi, lo, n2_hi, n2_lo = _make_mol(rm, qm)

        rbgs = SLOT_RBG[h]
        rowsel = np.concatenate(
            [np.arange(g * 128, (g + 1) * 128) for g in rbgs]
        )
        rowsL, rowsR = _aug_rows(hi, lo, n2_hi, n2_lo, rowsel)
        augs_np = np.ascontiguousarray(np.stack(rowsL).astype(bf))
        augm_np = np.ascontiguousarray(np.stack(rowsR).astype(bf))

        qi = qm[rowsel]  # [RB*128, NQ] slot-ordered
        q1_np = (
            qi.reshape(RB, 128, NQ).transpose(1, 0, 2).reshape(128, RB * NQ)
        ).astype(bf)
        q2_np = (2.0 * q1_np.astype(np.float32)).astype(bf)
        q12_np = np.ascontiguousarray(np.concatenate([q1_np, q2_np], axis=1))

        in_maps.append(
            {
                "augs": augs_np,
                "augm": augm_np,
                "q12": q12_np,
            }
        )
    return in_maps


def _host_diag_w(rm):
    """Replicate the device's diagonal weight w'_ii (f32 k-ordered accum)."""
    hi, lo, n2_hi, n2_lo = _make_mol(rm, None)
    rowsel = np.arange(NB)
    rowsL, rowsR = _aug_rows(hi, lo, n2_hi, n2_lo, rowsel)
    eps = np.zeros(NB, np.float32)
    for L, R in zip(rowsL, rowsR):
        eps = eps + L.astype(np.float32) * R.astype(np.float32)
    eps64 = eps.astype(np.float64)
    p = ((PC0 - eps64) * eps64 + PC1) * eps64 + PC2
    with np.errstate(divide="ignore"):
        y = 1.0 / np.sqrt(np.abs(ACT_SCALE * eps64))
    return np.minimum(y, np.maximum(p, 1.0))


def reduce_outputs(q, r, results):
    """Host-side gather: u[8,2048] per core -> pot[B].

    The device computes the diagonal with weight w'_ii = P(eps_ii)
    (eps_ii = deterministic rounding residue of s'_ii); replicate it
    here and subtract, then add the self term in f64.
    """
    q = np.asarray(q, np.float32)
    r = np.asarray(r, np.float32)
    TWOPI = 2.0 * np.pi
    pots = np.zeros(B, np.float64)
    for core in range(NCORES):
        b = core // 2
        u = results[core]["uout"].astype(np.float64)
        qm = q[b * NB : (b + 1) * NB].astype(np.float64)
        pots[b] += (u * qm.T).sum()
    for b in range(B):
        rm = r[b * NB : (b + 1) * NB]
        qm = q[b * NB : (b + 1) * NB].astype(np.float64)
        wdiag = _host_diag_w(rm)
        pots[b] -= ((qm**2).sum(1) * wdiag).sum()
    pots = pots / KAPPA / (4.0 * np.pi)
    for b in range(B):
        qm = q[b * NB : (b + 1) * NB].astype(np.float64)
        pots[b] += (qm**2).sum() / ((2.0 * np.pi) ** 1.5)
    return (pots * NORM_FACTOR).astype(np.float32)


def kernel(q, r, batch):
    global _compiled
    if _compiled is None:
        _compiled = build_nc()
    from concourse import bass_utils

    in_maps = make_in_maps(q, r)
    last_err = None
    for attempt in range(3):
        try:
            res = bass_utils.run_bass_kernel_spmd(
                _compiled, in_maps, core_ids=list(range(NCORES))
            )
            return reduce_outputs(q, r, res.results)
        except Exception as e:  # transient device errors: back off and retry
            last_err = e
            import time

            time.sleep(15 * (attempt + 1))
    raise last_err
